# revision 38
# baseline (speedup 1.0000x reference)
"""MoE router kernel for Trainium2 (8 NeuronCores, SPMD).

Computes, for X [16384, 2048] f32, W [64, 2048] f32, b [64] f32:
  router_logits    [T, 64] f32   = X @ W.T + b
  router_weights   [T, 2]  f32   = renormalized top-2 softmax probs
  selected_experts [T, 2]  i32   = top-2 expert indices
  expert_mask      [T,2,64] i32  = one-hot of selected_experts

Sharding: data-parallel over tokens (2048 tokens/core); gate weight
replicated.

Device-side strategy:
  - Host pre-transposes X to [H, T] and splits fp32 into two fp16 parts
    (x1 = fp16(X), x2 = fp16(X - x1)); likewise W into w1/w2. The PE
    matmul then runs at 1 cycle/row (vs 4 for fp32) with ~fp32-exact
    results via 4 cross terms:
       (x1+x2) @ (w1+w2) = x1w1 + x2w2 + x1w2 + x2w1
    The terms are packed into two stationary operands
       S1 = [w1T | w2T]  -> psum rows 0:64 += x1w1, rows 64:128 += x1w2
       S2 = [w2T | w1T]  -> psum rows 0:64 += x2w2, rows 64:128 += x2w1
    so logitsT = psum[0:64] + psum[64:128] after accumulating all 16
    h-chunks (PSUM fp32).
  - logitsT [64, tok] is transposed back to [tok, 64] via PE (identity),
    bias added on DVE, then DVE max/max_index give the top-2 values and
    indices per token; weights = softmax over the top-2 logits (which
    equals the reference's renormalized top-2 of the full softmax);
    expert mask via is_equal against an iota row.
"""

import numpy as np

T, H, E, TOPK = 16384, 2048, 64, 2
NCORES = 8
TL = T // NCORES        # tokens per core (2048)
NK = H // 128           # h-chunks (16)
NG = 4                  # token groups per core
GT = TL // NG           # tokens per group (512)
NT = TL // 128          # 128-token tiles per core (16)
TPG = GT // 128         # tiles per group (4)

_CACHE = {}


def _build_nc():
    import concourse.bass as bass
    import concourse.bacc as bacc
    import concourse.tile as tile
    from concourse import mybir

    f32 = mybir.dt.float32
    f16 = mybir.dt.float16
    i32 = mybir.dt.int32
    u32 = mybir.dt.uint32

    nc = bacc.Bacc(None, target_bir_lowering=False, debug=False)

    x1t = nc.dram_tensor("x1t", [H, TL], f16, kind="ExternalInput")
    x2t = nc.dram_tensor("x2t", [H, TL], f16, kind="ExternalInput")
    s1 = nc.dram_tensor("s1", [H, 128], f16, kind="ExternalInput")
    s2 = nc.dram_tensor("s2", [H, 128], f16, kind="ExternalInput")
    bias = nc.dram_tensor("bias", [E, 1], f32, kind="ExternalInput")
    ident = nc.dram_tensor("ident", [E, E], f32, kind="ExternalInput")

    logits_o = nc.dram_tensor("logits", [TL, E], f32, kind="ExternalOutput")
    weights_o = nc.dram_tensor("weights", [TL, TOPK], f32, kind="ExternalOutput")
    experts_o = nc.dram_tensor("experts", [TL, TOPK], i32, kind="ExternalOutput")
    mask_o = nc.dram_tensor("mask", [TL, TOPK * E], i32, kind="ExternalOutput")

    with tile.TileContext(nc) as tc:
        with (
            tc.tile_pool(name="const", bufs=1) as constp,
            tc.tile_pool(name="slabs", bufs=2) as slabp,
            tc.tile_pool(name="mm", bufs=2, space=bass.MemorySpace.PSUM) as mmpool,
            tc.tile_pool(name="tp", bufs=2, space=bass.MemorySpace.PSUM) as tppool,
            tc.tile_pool(name="ep", bufs=3) as ep,
            tc.tile_pool(name="stage", bufs=1) as stage,
        ):
            # ---- constants (ACT ring: keeps the SP ring free for x slabs) ----
            s1_sb = constp.tile([128, NK, 128], f16, name="s1", tag="s1")
            nc.scalar.dma_start(s1_sb[:], s1[:, :].rearrange("(k p) m -> p k m", p=128))
            s2_sb = constp.tile([128, NK, 128], f16, name="s2", tag="s2")
            nc.scalar.dma_start(s2_sb[:], s2[:, :].rearrange("(k p) m -> p k m", p=128))
            bias_sb = constp.tile([E, 1], f32, name="bias", tag="bias")
            nc.scalar.dma_start(bias_sb[:], bias[:, :])
            ident_sb = constp.tile([E, E], f32, name="ident", tag="ident")
            nc.scalar.dma_start(ident_sb[:], ident[:, :])

            # ---- stage buffers for the small outputs ----
            max_stage = stage.tile([128, NT, 8], f32, name="max8", tag="max8")
            idx_stage = stage.tile([128, NT, 8], u32, name="idx8", tag="idx8")
            d_stage = stage.tile([128, NT, TOPK], f32, name="d", tag="d")
            e_stage = stage.tile([128, NT, TOPK], f32, name="e", tag="e")
            s_stage = stage.tile([128, NT], f32, name="s", tag="s")
            r_stage = stage.tile([128, NT, 1], f32, name="r", tag="r")
            w_stage = stage.tile([128, NT, TOPK], f32, name="w", tag="w")

            # ---- token-group-major streaming: each group's [H, GT] slab
            # arrives as 4 h-quarter DMAs; the group's PSUM completes as
            # soon as its slab is in, so its epilogue overlaps the next
            # group's matmuls ----
            NQ = 4
            KPQ = NK // NQ  # h-chunks per quarter (4)

            def load_group(g):
                """Issue the 8 quarter-slab DMAs for group g, alternating the
                SP HWDGE ring and the SWDGE path so the two streams deliver
                each group's quarters in lockstep."""
                xq1, xq2 = [], []
                for q in range(NQ):
                    r0 = q * KPQ * 128
                    r1 = (q + 1) * KPQ * 128
                    eng_a = nc.sync if q % 2 == 0 else nc.gpsimd
                    eng_b = nc.gpsimd if q % 2 == 0 else nc.sync
                    ta = slabp.tile([128, KPQ, GT], f16, name=f"x1q{q}", tag=f"x1q{q}")
                    eng_a.dma_start(
                        ta[:],
                        x1t[r0:r1, g * GT:(g + 1) * GT].rearrange("(k p) c -> p k c", p=128),
                    )
                    xq1.append(ta)
                    tb = slabp.tile([128, KPQ, GT], f16, name=f"x2q{q}", tag=f"x2q{q}")
                    eng_b.dma_start(
                        tb[:],
                        x2t[r0:r1, g * GT:(g + 1) * GT].rearrange("(k p) c -> p k c", p=128),
                    )
                    xq2.append(tb)
                return xq1, xq2

            nxt = load_group(0)
            for g in range(NG):
                xq1, xq2 = nxt
                if g + 1 < NG:
                    nxt = load_group(g + 1)

                psum = mmpool.tile([128, GT], f32, name="mm", tag="mm")
                for k in range(NK):
                    nc.tensor.matmul(
                        psum[:, :], s1_sb[:, k, :], xq1[k // KPQ][:, k % KPQ, :],
                        start=(k == 0), stop=False, skip_group_check=True,
                    )
                    nc.tensor.matmul(
                        psum[:, :], s2_sb[:, k, :], xq2[k // KPQ][:, k % KPQ, :],
                        start=False, stop=(k == NK - 1), skip_group_check=True,
                    )

                # ---- group epilogue ----
                # lo half + bias (per-partition: experts are on partitions here)
                ltA = ep.tile([E, GT], f32, name="ltA", tag="ltA")
                nc.scalar.activation(
                    ltA[:, :], psum[0:E, :],
                    mybir.ActivationFunctionType.Identity, bias=bias_sb[:, :],
                )
                ltB = ep.tile([E, GT], f32, name="ltB", tag="ltB")
                nc.scalar.copy(ltB[:, :], psum[E:2 * E, :])
                for j in range(TPG):
                    t = g * TPG + j
                    # transpose both packed halves [64, 128] -> [128, 64],
                    # accumulating in PSUM: (lo + hi)^T = lo^T + hi^T
                    pt = tppool.tile([128, E], f32, name="pt", tag="pt")
                    nc.tensor.matmul(
                        pt[:, :], ltA[:, j * 128:(j + 1) * 128], ident_sb[:, :],
                        is_transpose=True, start=True, stop=False, skip_group_check=True,
                    )
                    nc.tensor.matmul(
                        pt[:, :], ltB[:, j * 128:(j + 1) * 128], ident_sb[:, :],
                        is_transpose=True, start=False, stop=True, skip_group_check=True,
                    )
                    # move PSUM -> SBUF on ACT
                    lg = ep.tile([128, E], f32, name="lg", tag="lg")
                    nc.scalar.copy(lg[:, :], pt[:, :])
                    nc.scalar.dma_start(logits_o[t * 128:(t + 1) * 128, :], lg[:, :])
                    # top-8 values + indices (we use the first 2)
                    nc.vector.max(max_stage[:, t, :], lg[:, :])
                    nc.vector.max_index(idx_stage[:, t, :], max_stage[:, t, :], lg[:, :])
                    # one-hot expert mask by value match against the top-2
                    # logits (exact fp32 equality with MAX8's outputs), one
                    # TensorTensor op for both k slots
                    mk = ep.tile([128, TOPK, E], i32, name="mk", tag="mk")
                    nc.vector.tensor_tensor(
                        mk[:, :, :],
                        lg[:, :].unsqueeze(1).to_broadcast([128, TOPK, E]),
                        max_stage[:, t, 0:TOPK].unsqueeze(2).to_broadcast([128, TOPK, E]),
                        mybir.AluOpType.is_equal,
                    )
                    nc.scalar.dma_start(
                        mask_o[t * 128:(t + 1) * 128, :],
                        mk[:, :, :].rearrange("p a b -> p (a b)"),
                    )

            # ---- routing weights for all tiles at once ----
            # d = top2 - l1 (broadcast l1 over the pair)
            nc.vector.tensor_tensor(
                d_stage[:, :, :],
                max_stage[:, :, 0:TOPK],
                max_stage[:, :, 0:1].to_broadcast([128, NT, TOPK]),
                mybir.AluOpType.subtract,
            )
            nc.scalar.activation(
                e_stage[:, :, :], d_stage[:, :, :], mybir.ActivationFunctionType.Exp,
            )
            nc.vector.tensor_add(s_stage[:, :], e_stage[:, :, 0], e_stage[:, :, 1])
            nc.vector.reciprocal(r_stage[:, :, 0], s_stage[:, :])
            nc.vector.tensor_tensor(
                w_stage[:, :, :], e_stage[:, :, :],
                r_stage[:, :, :].to_broadcast([128, NT, TOPK]),
                mybir.AluOpType.mult,
            )
            nc.scalar.dma_start(
                weights_o[:, :].rearrange("(t p) k -> p t k", p=128), w_stage[:, :, :]
            )
            nc.scalar.dma_start(
                experts_o[:, :].rearrange("(t p) k -> p t k", p=128),
                idx_stage[:, :, 0:TOPK].bitcast(i32),
            )

    nc.finalize()
    return nc


def _get_nc():
    if "nc" not in _CACHE:
        _CACHE["nc"] = _build_nc()
    return _CACHE["nc"]


def _prep_inputs(X, W, b):
    X = np.asarray(X, dtype=np.float32)
    W = np.asarray(W, dtype=np.float32)
    b = np.asarray(b, dtype=np.float32)

    x1 = X.astype(np.float16)
    x2 = (X - x1.astype(np.float32)).astype(np.float16)
    x1t = np.ascontiguousarray(x1.T)   # [H, T]
    x2t = np.ascontiguousarray(x2.T)

    w1 = W.astype(np.float16)
    w2 = (W - w1.astype(np.float32)).astype(np.float16)
    w1t = np.ascontiguousarray(w1.T)   # [H, E]
    w2t = np.ascontiguousarray(w2.T)
    s1 = np.concatenate([w1t, w2t], axis=1)  # [H, 128]
    s2 = np.concatenate([w2t, w1t], axis=1)

    bias = b.reshape(E, 1).astype(np.float32)
    ident = np.eye(E, dtype=np.float32)

    in_maps = []
    for c in range(NCORES):
        sl = slice(c * TL, (c + 1) * TL)
        in_maps.append({
            "x1t": x1t[:, sl],
            "x2t": x2t[:, sl],
            "s1": s1,
            "s2": s2,
            "bias": bias,
            "ident": ident,
        })
    return in_maps


def _assemble(results):
    logits = np.concatenate([r["logits"] for r in results], axis=0)
    weights = np.concatenate([r["weights"] for r in results], axis=0)
    experts = np.concatenate([r["experts"] for r in results], axis=0)
    mask = np.concatenate([r["mask"] for r in results], axis=0)
    return (
        logits.astype(np.float32, copy=False),
        weights.astype(np.float32, copy=False),
        experts.astype(np.int32, copy=False),
        mask.reshape(T, TOPK, E).astype(np.int32, copy=False),
    )


def kernel(X, W, b, **_unused):
    from concourse.bass_utils import run_bass_kernel_spmd

    nc = _get_nc()
    in_maps = _prep_inputs(X, W, b)
    res = run_bass_kernel_spmd(nc, in_maps, core_ids=list(range(NCORES)))
    return _assemble(res.results)


# revision 45
# speedup vs baseline: 1.2308x; 1.2308x over previous
"""MoE router kernel for Trainium2 (8 NeuronCores, SPMD).

Computes, for X [16384, 2048] f32, W [64, 2048] f32, b [64] f32:
  router_logits    [T, 64] f32   = X @ W.T + b
  router_weights   [T, 2]  f32   = renormalized top-2 softmax probs
  selected_experts [T, 2]  i32   = top-2 expert indices
  expert_mask      [T,2,64] i32  = one-hot of selected_experts

Sharding: data-parallel over tokens (2048 tokens/core); gate weight
replicated.

Device-side strategy:
  - Host pre-transposes X to [H, T] and splits fp32 into two fp16 parts
    (x1 = fp16(X), x2 = fp16(X - x1)); likewise W into w1/w2. The PE
    matmul then runs at 1 cycle/row (vs 4 for fp32) with ~fp32-exact
    results via 4 cross terms:
       (x1+x2) @ (w1+w2) = x1w1 + x2w2 + x1w2 + x2w1
    The terms are packed into two stationary operands
       S1 = [w1T | w2T]  -> psum rows 0:64 += x1w1, rows 64:128 += x1w2
       S2 = [w2T | w1T]  -> psum rows 0:64 += x2w2, rows 64:128 += x2w1
    so logitsT = psum[0:64] + psum[64:128] after accumulating all 16
    h-chunks (PSUM fp32).
  - logitsT [64, tok] is transposed back to [tok, 64] via PE (identity),
    bias added on DVE, then DVE max/max_index give the top-2 values and
    indices per token; weights = softmax over the top-2 logits (which
    equals the reference's renormalized top-2 of the full softmax);
    expert mask via is_equal against an iota row.
"""

import numpy as np

T, H, E, TOPK = 16384, 2048, 64, 2
NCORES = 8
TL = T // NCORES        # tokens per core (2048)
NK = H // 128           # h-chunks (16)
NG = 4                  # token groups per core
GT = TL // NG           # tokens per group (512)
NT = TL // 128          # 128-token tiles per core (16)
TPG = GT // 128         # tiles per group (4)

_CACHE = {}


def _build_nc():
    import concourse.bass as bass
    import concourse.bacc as bacc
    import concourse.tile as tile
    from concourse import mybir

    f32 = mybir.dt.float32
    f16 = mybir.dt.float16
    i32 = mybir.dt.int32
    u32 = mybir.dt.uint32

    nc = bacc.Bacc(None, target_bir_lowering=False, debug=False)

    x1t = nc.dram_tensor("x1t", [H, TL], f16, kind="ExternalInput")
    x2t = nc.dram_tensor("x2t", [H, TL], f16, kind="ExternalInput")
    s1 = nc.dram_tensor("s1", [H, 128], f16, kind="ExternalInput")
    s2 = nc.dram_tensor("s2", [H, 128], f16, kind="ExternalInput")
    bias = nc.dram_tensor("bias", [E, E], f32, kind="ExternalInput")
    ident = nc.dram_tensor("ident", [E, E], f32, kind="ExternalInput")

    logits_o = nc.dram_tensor("logits", [TL, E], f32, kind="ExternalOutput")
    # weights/experts leave the device in SBUF-dump layout [p, t, k];
    # the host reorders to [t*128+p, k]
    weights_o = nc.dram_tensor("weights", [128, NT * TOPK], f32, kind="ExternalOutput")
    experts_o = nc.dram_tensor("experts", [128, NT * TOPK], i32, kind="ExternalOutput")
    mask_o = nc.dram_tensor("mask", [TL, TOPK * E], i32, kind="ExternalOutput")

    with tile.TileContext(nc) as tc:
        with (
            tc.tile_pool(name="const", bufs=1) as constp,
            tc.tile_pool(name="slabs", bufs=2) as slabp,
            tc.tile_pool(name="mm", bufs=2, space=bass.MemorySpace.PSUM) as mmpool,
            tc.tile_pool(name="tp", bufs=2, space=bass.MemorySpace.PSUM) as tppool,
            tc.tile_pool(name="ep", bufs=3) as ep,
            tc.tile_pool(name="stage", bufs=1) as stage,
        ):
            # ---- constants (SWDGE, ahead of the slab stream) ----
            s1_sb = constp.tile([128, NK, 128], f16, name="s1", tag="s1")
            nc.gpsimd.dma_start(s1_sb[:], s1[:, :].rearrange("(k p) m -> p k m", p=128))
            s2_sb = constp.tile([128, NK, 128], f16, name="s2", tag="s2")
            nc.gpsimd.dma_start(s2_sb[:], s2[:, :].rearrange("(k p) m -> p k m", p=128))
            # bias arrives replicated [64, 64] (256B rows); column 0 is used
            bias_sb = constp.tile([E, E], f32, name="bias", tag="bias")
            nc.gpsimd.dma_start(bias_sb[:], bias[:, :])
            ident_sb = constp.tile([E, E], f32, name="ident", tag="ident")
            nc.gpsimd.dma_start(ident_sb[:], ident[:, :])

            # ---- stage buffers for the small outputs ----
            max_stage = stage.tile([128, NT, 8], f32, name="max8", tag="max8")
            idx_stage = stage.tile([128, NT, 8], u32, name="idx8", tag="idx8")
            d_stage = stage.tile([128, NT, TOPK], f32, name="d", tag="d")
            e_stage = stage.tile([128, NT, TOPK], f32, name="e", tag="e")
            s_stage = stage.tile([128, NT], f32, name="s", tag="s")
            r_stage = stage.tile([128, NT, 1], f32, name="r", tag="r")
            w_stage = stage.tile([128, NT, TOPK], f32, name="w", tag="w")

            # ---- token-group-major streaming: each group's [H, GT] slab
            # arrives as 4 h-quarter DMAs; the group's PSUM completes as
            # soon as its slab is in, so its epilogue overlaps the next
            # group's matmuls ----
            NQ = 4
            KPQ = NK // NQ  # h-chunks per quarter (4)

            def load_group(g):
                """Issue the 8 quarter-slab DMAs for group g, alternating the
                SP HWDGE ring and the SWDGE path so the two streams deliver
                each group's quarters in lockstep."""
                xq1, xq2 = [], []
                for q in range(NQ):
                    r0 = q * KPQ * 128
                    r1 = (q + 1) * KPQ * 128
                    ta = slabp.tile([128, KPQ, GT], f16, name=f"x1q{q}", tag=f"x1q{q}")
                    nc.gpsimd.dma_start(
                        ta[:],
                        x1t[r0:r1, g * GT:(g + 1) * GT].rearrange("(k p) c -> p k c", p=128),
                    )
                    xq1.append(ta)
                    tb = slabp.tile([128, KPQ, GT], f16, name=f"x2q{q}", tag=f"x2q{q}")
                    nc.gpsimd.dma_start(
                        tb[:],
                        x2t[r0:r1, g * GT:(g + 1) * GT].rearrange("(k p) c -> p k c", p=128),
                    )
                    xq2.append(tb)
                return xq1, xq2

            nxt = load_group(0)
            for g in range(NG):
                xq1, xq2 = nxt
                if g + 1 < NG:
                    nxt = load_group(g + 1)

                psum = mmpool.tile([128, GT], f32, name="mm", tag="mm")
                for k in range(NK):
                    nc.tensor.matmul(
                        psum[:, :], s1_sb[:, k, :], xq1[k // KPQ][:, k % KPQ, :],
                        start=(k == 0), stop=False, skip_group_check=True,
                    )
                    nc.tensor.matmul(
                        psum[:, :], s2_sb[:, k, :], xq2[k // KPQ][:, k % KPQ, :],
                        start=False, stop=(k == NK - 1), skip_group_check=True,
                    )

                # ---- group epilogue ----
                # lo half + bias (per-partition: experts are on partitions here)
                ltA = ep.tile([E, GT], f32, name="ltA", tag="ltA")
                nc.scalar.activation(
                    ltA[:, :], psum[0:E, :],
                    mybir.ActivationFunctionType.Identity, bias=bias_sb[:, 0:1],
                )
                ltB = ep.tile([E, GT], f32, name="ltB", tag="ltB")
                nc.scalar.copy(ltB[:, :], psum[E:2 * E, :])
                for j in range(TPG):
                    t = g * TPG + j
                    # transpose both packed halves [64, 128] -> [128, 64],
                    # accumulating in PSUM: (lo + hi)^T = lo^T + hi^T
                    pt = tppool.tile([128, E], f32, name="pt", tag="pt")
                    nc.tensor.matmul(
                        pt[:, :], ltA[:, j * 128:(j + 1) * 128], ident_sb[:, :],
                        is_transpose=True, start=True, stop=False, skip_group_check=True,
                    )
                    nc.tensor.matmul(
                        pt[:, :], ltB[:, j * 128:(j + 1) * 128], ident_sb[:, :],
                        is_transpose=True, start=False, stop=True, skip_group_check=True,
                    )
                    # move PSUM -> SBUF on ACT
                    lg = ep.tile([128, E], f32, name="lg", tag="lg")
                    nc.scalar.copy(lg[:, :], pt[:, :])
                    nc.scalar.dma_start(logits_o[t * 128:(t + 1) * 128, :], lg[:, :])
                    # top-8 values + indices (we use the first 2)
                    nc.vector.max(max_stage[:, t, :], lg[:, :])
                    nc.vector.max_index(idx_stage[:, t, :], max_stage[:, t, :], lg[:, :])
                    # one-hot expert mask by value match against the top-2
                    # logits (exact fp32 equality with MAX8's outputs), one
                    # TensorTensor op for both k slots
                    mk = ep.tile([128, TOPK, E], i32, name="mk", tag="mk")
                    nc.vector.tensor_tensor(
                        mk[:, :, :],
                        lg[:, :].unsqueeze(1).to_broadcast([128, TOPK, E]),
                        max_stage[:, t, 0:TOPK].unsqueeze(2).to_broadcast([128, TOPK, E]),
                        mybir.AluOpType.is_equal,
                    )
                    nc.scalar.dma_start(
                        mask_o[t * 128:(t + 1) * 128, :],
                        mk[:, :, :].rearrange("p a b -> p (a b)"),
                    )

            # ---- routing weights for all tiles at once ----
            # d = top2 - l1 (broadcast l1 over the pair)
            nc.vector.tensor_tensor(
                d_stage[:, :, :],
                max_stage[:, :, 0:TOPK],
                max_stage[:, :, 0:1].to_broadcast([128, NT, TOPK]),
                mybir.AluOpType.subtract,
            )
            nc.scalar.activation(
                e_stage[:, :, :], d_stage[:, :, :], mybir.ActivationFunctionType.Exp,
            )
            nc.vector.tensor_add(s_stage[:, :], e_stage[:, :, 0], e_stage[:, :, 1])
            nc.vector.reciprocal(r_stage[:, :, 0], s_stage[:, :])
            nc.vector.tensor_tensor(
                w_stage[:, :, :], e_stage[:, :, :],
                r_stage[:, :, :].to_broadcast([128, NT, TOPK]),
                mybir.AluOpType.mult,
            )
            # compact the top-2 indices (u32 -> i32 cast) so the dump DMA
            # moves 128B-contiguous rows instead of 8B fragments
            ex_stage = stage.tile([128, NT, TOPK], i32, name="ex", tag="ex")
            nc.vector.tensor_copy(ex_stage[:, :, :], idx_stage[:, :, 0:TOPK])
            nc.scalar.dma_start(
                weights_o[:, :].rearrange("p (t k) -> p t k", k=TOPK), w_stage[:, :, :]
            )
            nc.scalar.dma_start(
                experts_o[:, :].rearrange("p (t k) -> p t k", k=TOPK), ex_stage[:, :, :]
            )

    nc.finalize()
    return nc


def _get_nc():
    if "nc" not in _CACHE:
        _CACHE["nc"] = _build_nc()
    return _CACHE["nc"]


def _prep_inputs(X, W, b):
    X = np.asarray(X, dtype=np.float32)
    W = np.asarray(W, dtype=np.float32)
    b = np.asarray(b, dtype=np.float32)

    x1 = X.astype(np.float16)
    x2 = (X - x1.astype(np.float32)).astype(np.float16)
    x1t = np.ascontiguousarray(x1.T)   # [H, T]
    x2t = np.ascontiguousarray(x2.T)

    w1 = W.astype(np.float16)
    w2 = (W - w1.astype(np.float32)).astype(np.float16)
    w1t = np.ascontiguousarray(w1.T)   # [H, E]
    w2t = np.ascontiguousarray(w2.T)
    s1 = np.concatenate([w1t, w2t], axis=1)  # [H, 128]
    s2 = np.concatenate([w2t, w1t], axis=1)

    bias = np.tile(b.reshape(E, 1).astype(np.float32), (1, E))
    ident = np.eye(E, dtype=np.float32)

    in_maps = []
    for c in range(NCORES):
        sl = slice(c * TL, (c + 1) * TL)
        in_maps.append({
            "x1t": x1t[:, sl],
            "x2t": x2t[:, sl],
            "s1": s1,
            "s2": s2,
            "bias": bias,
            "ident": ident,
        })
    return in_maps


def _assemble(results):
    def undump(a):
        # [128, NT*K] dump layout -> [TL, K]
        return np.ascontiguousarray(
            np.transpose(a.reshape(128, NT, TOPK), (1, 0, 2)).reshape(TL, TOPK)
        )

    logits = np.concatenate([r["logits"] for r in results], axis=0)
    weights = np.concatenate([undump(r["weights"]) for r in results], axis=0)
    experts = np.concatenate([undump(r["experts"]) for r in results], axis=0)
    mask = np.concatenate([r["mask"] for r in results], axis=0)
    return (
        logits.astype(np.float32, copy=False),
        weights.astype(np.float32, copy=False),
        experts.astype(np.int32, copy=False),
        mask.reshape(T, TOPK, E).astype(np.int32, copy=False),
    )


def kernel(X, W, b, **_unused):
    from concourse.bass_utils import run_bass_kernel_spmd

    nc = _get_nc()
    in_maps = _prep_inputs(X, W, b)
    res = run_bass_kernel_spmd(nc, in_maps, core_ids=list(range(NCORES)))
    return _assemble(res.results)


# revision 51
# speedup vs baseline: 1.2938x; 1.0512x over previous
"""MoE router kernel for Trainium2 (8 NeuronCores, SPMD).

Computes, for X [16384, 2048] f32, W [64, 2048] f32, b [64] f32:
  router_logits    [T, 64] f32   = X @ W.T + b
  router_weights   [T, 2]  f32   = renormalized top-2 softmax probs
  selected_experts [T, 2]  i32   = top-2 expert indices
  expert_mask      [T,2,64] i32  = one-hot of selected_experts

Sharding: data-parallel over tokens (2048 tokens/core); gate weight
replicated.

Device-side strategy:
  - Host pre-transposes X to [H, T] and splits fp32 into two fp16 parts
    (x1 = fp16(X), x2 = fp16(X - x1)); likewise W into w1/w2. The PE
    matmul then runs at 1 cycle/row (vs 4 for fp32) with ~fp32-exact
    results via 4 cross terms:
       (x1+x2) @ (w1+w2) = x1w1 + x2w2 + x1w2 + x2w1
    The terms are packed into two stationary operands
       S1 = [w1T | w2T]  -> psum rows 0:64 += x1w1, rows 64:128 += x1w2
       S2 = [w2T | w1T]  -> psum rows 0:64 += x2w2, rows 64:128 += x2w1
    so logitsT = psum[0:64] + psum[64:128] after accumulating all 16
    h-chunks (PSUM fp32).
  - logitsT [64, tok] is transposed back to [tok, 64] via PE (identity),
    bias added on DVE, then DVE max/max_index give the top-2 values and
    indices per token; weights = softmax over the top-2 logits (which
    equals the reference's renormalized top-2 of the full softmax);
    expert mask via is_equal against an iota row.
"""

import numpy as np

T, H, E, TOPK = 16384, 2048, 64, 2
NCORES = 8
TL = T // NCORES        # tokens per core (2048)
NK = H // 128           # h-chunks (16)
NG = 4                  # token groups per core
GT = TL // NG           # tokens per group (512)
NT = TL // 128          # 128-token tiles per core (16)
TPG = GT // 128         # tiles per group (4)

_CACHE = {}


def _build_nc():
    import concourse.bass as bass
    import concourse.bacc as bacc
    import concourse.tile as tile
    from concourse import mybir

    f32 = mybir.dt.float32
    f16 = mybir.dt.float16
    i32 = mybir.dt.int32
    u32 = mybir.dt.uint32

    nc = bacc.Bacc(None, target_bir_lowering=False, debug=False)

    x1t = nc.dram_tensor("x1t", [H, TL], f16, kind="ExternalInput")
    x2t = nc.dram_tensor("x2t", [H, TL], f16, kind="ExternalInput")
    s1 = nc.dram_tensor("s1", [H, 128], f16, kind="ExternalInput")
    s2 = nc.dram_tensor("s2", [H, 128], f16, kind="ExternalInput")
    bias = nc.dram_tensor("bias", [E, E], f32, kind="ExternalInput")
    ident = nc.dram_tensor("ident", [E, E], f32, kind="ExternalInput")

    logits_o = nc.dram_tensor("logits", [TL, E], f32, kind="ExternalOutput")
    # weights/experts leave the device in SBUF-dump layout [p, t, k];
    # the host reorders to [t*128+p, k]
    weights_o = nc.dram_tensor("weights", [128, NT * TOPK], f32, kind="ExternalOutput")
    experts_o = nc.dram_tensor("experts", [128, NT * TOPK], i32, kind="ExternalOutput")
    mask_o = nc.dram_tensor("mask", [TL, TOPK * E], i32, kind="ExternalOutput")

    with tile.TileContext(nc) as tc:
        with (
            tc.tile_pool(name="const", bufs=1) as constp,
            tc.tile_pool(name="slabs", bufs=2) as slabp,
            tc.tile_pool(name="mm", bufs=3, space=bass.MemorySpace.PSUM) as mmpool,
            tc.tile_pool(name="tp", bufs=2, space=bass.MemorySpace.PSUM) as tppool,
            tc.tile_pool(name="ep", bufs=3) as ep,
            tc.tile_pool(name="stage", bufs=1) as stage,
        ):
            # ---- constants: quarter-interleaved with group 0's slabs below,
            # so the first matmuls unblock as early as possible ----
            s1_sb = constp.tile([128, NK, 128], f16, name="s1", tag="s1")
            s2_sb = constp.tile([128, NK, 128], f16, name="s2", tag="s2")
            bias_sb = constp.tile([E, E], f32, name="bias", tag="bias")
            ident_sb = constp.tile([E, E], f32, name="ident", tag="ident")

            # ---- stage buffers for the small outputs ----
            max_stage = stage.tile([128, NT, 8], f32, name="max8", tag="max8")
            idx_stage = stage.tile([128, NT, 8], u32, name="idx8", tag="idx8")
            d_stage = stage.tile([128, NT, TOPK], f32, name="d", tag="d")
            e_stage = stage.tile([128, NT, TOPK], f32, name="e", tag="e")
            s_stage = stage.tile([128, NT], f32, name="s", tag="s")
            r_stage = stage.tile([128, NT, 1], f32, name="r", tag="r")
            w_stage = stage.tile([128, NT, TOPK], f32, name="w", tag="w")

            # ---- token-group-major streaming: each group's [H, GT] slab
            # arrives as 4 h-quarter DMAs; the group's PSUM completes as
            # soon as its slab is in, so its epilogue overlaps the next
            # group's matmuls ----
            NQ = 4
            KPQ = NK // NQ  # h-chunks per quarter (4)

            def load_group(g, with_consts=False):
                """Queue the 8 quarter-slab SWDGE DMAs for group g in
                consumption order; for group 0 interleave the stationary
                quarters ahead of each slab quarter."""
                xq1, xq2 = [], []
                for q in range(NQ):
                    r0 = q * KPQ * 128
                    r1 = (q + 1) * KPQ * 128
                    if with_consts:
                        nc.gpsimd.dma_start(
                            s1_sb[:, q * KPQ:(q + 1) * KPQ, :],
                            s1[r0:r1, :].rearrange("(k p) m -> p k m", p=128),
                        )
                        nc.gpsimd.dma_start(
                            s2_sb[:, q * KPQ:(q + 1) * KPQ, :],
                            s2[r0:r1, :].rearrange("(k p) m -> p k m", p=128),
                        )
                    ta = slabp.tile([128, KPQ, GT], f16, name=f"x1q{q}", tag=f"x1q{q}")
                    nc.gpsimd.dma_start(
                        ta[:],
                        x1t[r0:r1, g * GT:(g + 1) * GT].rearrange("(k p) c -> p k c", p=128),
                    )
                    xq1.append(ta)
                    tb = slabp.tile([128, KPQ, GT], f16, name=f"x2q{q}", tag=f"x2q{q}")
                    nc.gpsimd.dma_start(
                        tb[:],
                        x2t[r0:r1, g * GT:(g + 1) * GT].rearrange("(k p) c -> p k c", p=128),
                    )
                    xq2.append(tb)
                if with_consts:
                    nc.gpsimd.dma_start(bias_sb[:], bias[:, :])
                    nc.gpsimd.dma_start(ident_sb[:], ident[:, :])
                return xq1, xq2

            nxt = load_group(0, with_consts=True)
            for g in range(NG):
                xq1, xq2 = nxt
                if g + 1 < NG:
                    nxt = load_group(g + 1)

                psum = mmpool.tile([128, GT], f32, name="mm", tag="mm")
                for k in range(NK):
                    nc.tensor.matmul(
                        psum[:, :], s1_sb[:, k, :], xq1[k // KPQ][:, k % KPQ, :],
                        start=(k == 0), stop=False, skip_group_check=True,
                    )
                    nc.tensor.matmul(
                        psum[:, :], s2_sb[:, k, :], xq2[k // KPQ][:, k % KPQ, :],
                        start=False, stop=(k == NK - 1), skip_group_check=True,
                    )

                # ---- group epilogue ----
                # lo half + bias (per-partition: experts are on partitions here)
                ltA = ep.tile([E, GT], f32, name="ltA", tag="ltA")
                nc.scalar.activation(
                    ltA[:, :], psum[0:E, :],
                    mybir.ActivationFunctionType.Identity, bias=bias_sb[:, 0:1],
                )
                ltB = ep.tile([E, GT], f32, name="ltB", tag="ltB")
                nc.scalar.copy(ltB[:, :], psum[E:2 * E, :])
                for j in range(TPG):
                    t = g * TPG + j
                    # transpose both packed halves [64, 128] -> [128, 64],
                    # accumulating in PSUM: (lo + hi)^T = lo^T + hi^T
                    pt = tppool.tile([128, E], f32, name="pt", tag="pt")
                    nc.tensor.matmul(
                        pt[:, :], ltA[:, j * 128:(j + 1) * 128], ident_sb[:, :],
                        is_transpose=True, start=True, stop=False, skip_group_check=True,
                    )
                    nc.tensor.matmul(
                        pt[:, :], ltB[:, j * 128:(j + 1) * 128], ident_sb[:, :],
                        is_transpose=True, start=False, stop=True, skip_group_check=True,
                    )
                    # move PSUM -> SBUF on ACT
                    lg = ep.tile([128, E], f32, name="lg", tag="lg")
                    nc.scalar.copy(lg[:, :], pt[:, :])
                    nc.sync.dma_start(logits_o[t * 128:(t + 1) * 128, :], lg[:, :])
                    # top-8 values + indices (we use the first 2)
                    nc.vector.max(max_stage[:, t, :], lg[:, :])
                    nc.vector.max_index(idx_stage[:, t, :], max_stage[:, t, :], lg[:, :])
                    # one-hot expert mask by value match against the top-2
                    # logits (exact fp32 equality with MAX8's outputs), one
                    # TensorTensor op for both k slots
                    mk = ep.tile([128, TOPK, E], i32, name="mk", tag="mk")
                    nc.vector.tensor_tensor(
                        mk[:, :, :],
                        lg[:, :].unsqueeze(1).to_broadcast([128, TOPK, E]),
                        max_stage[:, t, 0:TOPK].unsqueeze(2).to_broadcast([128, TOPK, E]),
                        mybir.AluOpType.is_equal,
                    )
                    nc.sync.dma_start(
                        mask_o[t * 128:(t + 1) * 128, :],
                        mk[:, :, :].rearrange("p a b -> p (a b)"),
                    )

            # ---- routing weights for all tiles at once ----
            # d = top2 - l1 (broadcast l1 over the pair)
            nc.vector.tensor_tensor(
                d_stage[:, :, :],
                max_stage[:, :, 0:TOPK],
                max_stage[:, :, 0:1].to_broadcast([128, NT, TOPK]),
                mybir.AluOpType.subtract,
            )
            nc.scalar.activation(
                e_stage[:, :, :], d_stage[:, :, :], mybir.ActivationFunctionType.Exp,
            )
            nc.vector.tensor_add(s_stage[:, :], e_stage[:, :, 0], e_stage[:, :, 1])
            nc.vector.reciprocal(r_stage[:, :, 0], s_stage[:, :])
            nc.vector.tensor_tensor(
                w_stage[:, :, :], e_stage[:, :, :],
                r_stage[:, :, :].to_broadcast([128, NT, TOPK]),
                mybir.AluOpType.mult,
            )
            # compact the top-2 indices (u32 -> i32 cast) so the dump DMA
            # moves 128B-contiguous rows instead of 8B fragments
            ex_stage = stage.tile([128, NT, TOPK], i32, name="ex", tag="ex")
            nc.vector.tensor_copy(ex_stage[:, :, :], idx_stage[:, :, 0:TOPK])
            nc.sync.dma_start(
                weights_o[:, :].rearrange("p (t k) -> p t k", k=TOPK), w_stage[:, :, :]
            )
            nc.sync.dma_start(
                experts_o[:, :].rearrange("p (t k) -> p t k", k=TOPK), ex_stage[:, :, :]
            )

    nc.finalize()
    return nc


def _get_nc():
    if "nc" not in _CACHE:
        _CACHE["nc"] = _build_nc()
    return _CACHE["nc"]


def _prep_inputs(X, W, b):
    X = np.asarray(X, dtype=np.float32)
    W = np.asarray(W, dtype=np.float32)
    b = np.asarray(b, dtype=np.float32)

    x1 = X.astype(np.float16)
    x2 = (X - x1.astype(np.float32)).astype(np.float16)
    x1t = np.ascontiguousarray(x1.T)   # [H, T]
    x2t = np.ascontiguousarray(x2.T)

    w1 = W.astype(np.float16)
    w2 = (W - w1.astype(np.float32)).astype(np.float16)
    w1t = np.ascontiguousarray(w1.T)   # [H, E]
    w2t = np.ascontiguousarray(w2.T)
    s1 = np.concatenate([w1t, w2t], axis=1)  # [H, 128]
    s2 = np.concatenate([w2t, w1t], axis=1)

    bias = np.tile(b.reshape(E, 1).astype(np.float32), (1, E))
    ident = np.eye(E, dtype=np.float32)

    in_maps = []
    for c in range(NCORES):
        sl = slice(c * TL, (c + 1) * TL)
        in_maps.append({
            "x1t": x1t[:, sl],
            "x2t": x2t[:, sl],
            "s1": s1,
            "s2": s2,
            "bias": bias,
            "ident": ident,
        })
    return in_maps


def _assemble(results):
    def undump(a):
        # [128, NT*K] dump layout -> [TL, K]
        return np.ascontiguousarray(
            np.transpose(a.reshape(128, NT, TOPK), (1, 0, 2)).reshape(TL, TOPK)
        )

    logits = np.concatenate([r["logits"] for r in results], axis=0)
    weights = np.concatenate([undump(r["weights"]) for r in results], axis=0)
    experts = np.concatenate([undump(r["experts"]) for r in results], axis=0)
    mask = np.concatenate([r["mask"] for r in results], axis=0)
    return (
        logits.astype(np.float32, copy=False),
        weights.astype(np.float32, copy=False),
        experts.astype(np.int32, copy=False),
        mask.reshape(T, TOPK, E).astype(np.int32, copy=False),
    )


def kernel(X, W, b, **_unused):
    from concourse.bass_utils import run_bass_kernel_spmd

    nc = _get_nc()
    in_maps = _prep_inputs(X, W, b)
    res = run_bass_kernel_spmd(nc, in_maps, core_ids=list(range(NCORES)))
    return _assemble(res.results)
